# revision 1
# baseline (speedup 1.0000x reference)
"""Trainium2 Bass kernel for a dense transformer block (single-head attn + MLP).

Sharding: 8 cores; core c handles batch b=c//2, query-half h=c%2.
Each core computes K,V for all 2048 tokens of its batch (no collectives).
Host permutes tokens so each core's query tokens are always columns 0..1023
of its transposed input (SPMD uniform program).

Layout: activations kept transposed [C, T] (channels on partitions) so every
matmul feeds the PE directly.  LN stats via ones-matmuls on PE; per-token row
broadcasts via K=1 outer-product matmuls; softmax without max subtraction
(logits are ~N(0, 0.4^2): verified small); softmax denominator folded into
the y-eviction.  All matmuls in float32r (FP22, full PE rate at N>=256).
Attention scores are computed inside phase A while k^T is still in SBUF;
exp(att) and V spill to DRAM and stream back for the y/proj phase.  The MLP
streams each weight exactly once (H-halves, DMA-accumulated output).
"""

import numpy as np
import concourse.bass as bass
import concourse.mybir as mybir
import concourse.tile as tile
from concourse import bacc
from concourse.bass_utils import run_bass_kernel_spmd

F32 = mybir.dt.float32
F32R = mybir.dt.float32r
AF = mybir.ActivationFunctionType
ALU = mybir.AluOpType

P = 128
C = 1024        # n_embd
T = 2048        # key tokens per batch
TQ = 1024       # query tokens per core
H = 4096        # mlp hidden
CK = C // P     # 8
HK = H // P     # 32
S = T // P      # 16 key tiles
NCH = 512       # matmul moving-dim chunk
EPS = 1e-5
ATT_SCALE = 1.0 / 32.0   # 1/sqrt(C)

N_CORES = 8


def _build(reps=1, debug_taps=False):
    nc = bacc.Bacc()

    xT = nc.declare_dram_parameter("xT", [C, TQ], F32R, isOutput=False)
    w1qk = nc.declare_dram_parameter("w1qk", [2 * CK, P, C], F32R, isOutput=False)
    w1v = nc.declare_dram_parameter("w1v", [CK, P, C], F32R, isOutput=False)
    wp = nc.declare_dram_parameter("wp", [CK, P, C], F32R, isOutput=False)
    w2 = nc.declare_dram_parameter("w2", [HK, P, C], F32R, isOutput=False)
    wm = nc.declare_dram_parameter("wm", [CK, P, H], F32R, isOutput=False)
    c1q = nc.declare_dram_parameter("c1q", [CK, P], F32, isOutput=False)
    c1k = nc.declare_dram_parameter("c1k", [CK, P], F32, isOutput=False)
    c1vb = nc.declare_dram_parameter("c1vb", [P, C], F32, isOutput=False)
    bp = nc.declare_dram_parameter("bp", [CK, P], F32, isOutput=False)
    c2 = nc.declare_dram_parameter("c2", [HK, P], F32, isOutput=False)
    bm = nc.declare_dram_parameter("bm", [CK, P], F32, isOutput=False)
    onc = nc.declare_dram_parameter("onc", [P, 1], F32R, isOutput=False)
    onr = nc.declare_dram_parameter("onr", [1, P], F32R, isOutput=False)
    out_t = nc.declare_dram_parameter("out_t", [C, TQ], F32, isOutput=True)

    kv_self = nc.dram_tensor("kv_self", [2, TQ // P, P, TQ], F32R)
    kv_all = nc.dram_tensor("kv_all", [2, 2, TQ // P, P, TQ], F32R)
    if debug_taps:
        yT_d = nc.dram_tensor("yT_d", [P, CK, TQ], F32)
        x2_d = nc.dram_tensor("x2_d", [P, CK, TQ], F32)
        xh2_d = nc.dram_tensor("xh2_d", [P, CK, TQ], F32)
        gel_d = nc.dram_tensor("gel_d", [P, HK // 2, TQ], F32)
    attd = nc.dram_tensor("attd", [S, P, TQ], F32R)

    xT3 = xT.rearrange("(k p) t -> p k t", p=P)

    with tile.TileContext(nc) as tc:
        with (
            tc.tile_pool(name="glob", bufs=1) as gp,
            tc.tile_pool(name="ps", bufs=8, space="PSUM") as pp,
        ):
            def pst(pdim=P):
                return pp.tile([pdim, NCH], F32, tag="ps", name="ps")

            ones_col = gp.tile([P, 1], F32R)
            nc.sync.dma_start(ones_col[:], onc[:])
            ones_row = gp.tile([1, P], F32R)
            nc.sync.dma_start(ones_row[:], onr[:])
            c1q_t = gp.tile([P, CK], F32)
            nc.sync.dma_start(c1q_t[:], c1q.rearrange("j p -> p j"))
            c1k_t = gp.tile([P, CK], F32)
            nc.sync.dma_start(c1k_t[:], c1k.rearrange("j p -> p j"))
            bp_t = gp.tile([P, CK], F32)
            nc.sync.dma_start(bp_t[:], bp.rearrange("j p -> p j"))
            bm_t = gp.tile([P, CK], F32)
            nc.sync.dma_start(bm_t[:], bm.rearrange("j p -> p j"))
            c2_t = gp.tile([P, HK], F32)
            nc.sync.dma_start(c2_t[:], c2.rearrange("j p -> p j"))
            c1v_t = gp.tile([P, C], F32)
            nc.sync.dma_start(c1v_t[:], c1vb[:])
            recip_b = gp.tile([P, TQ], F32)
            eps_col = gp.tile([P, 1], F32)
            nc.vector.memset(eps_col[:], EPS)

            def ln_center(src3, width, r_b, mu_b, rbase, xc, r_col=None):
                """LN stats over channels of a transposed activation.
                Centers src3 into xc (xc may be src3: in-place); fills
                broadcast rows r_b/mu_b[:, rbase:rbase+width] and the f32r
                reciprocal-sigma row r_row_r[:, rbase:rbase+width].
                Callers fold the * r scaling into their PSUM evictions."""
                for sub in range(width // NCH):
                    lo = sub * NCH
                    mu_ps = pst(1)
                    s2_ps = pst(1)
                    for k in range(CK):
                        nc.tensor.matmul(mu_ps[:], ones_col[:],
                                         src3[:, k, lo:lo + NCH],
                                         start=(k == 0), stop=(k == CK - 1))
                    for k in range(CK):
                        sq = gp.tile([P, NCH], F32R, tag="sq", bufs=2)
                        nc.scalar.activation(
                            sq[:], src3[:, k, lo:lo + NCH].bitcast(F32),
                            AF.Square)
                        nc.tensor.matmul(s2_ps[:], ones_col[:], sq[:],
                                         start=(k == 0), stop=(k == CK - 1))
                    mu_sb = gp.tile([1, NCH], F32R, tag="murow", bufs=2)
                    nc.scalar.activation(mu_sb[:], mu_ps[:], AF.Copy,
                                         scale=1.0 / C)
                    musq = gp.tile([1, NCH], F32, tag="musq", bufs=2)
                    nc.scalar.activation(musq[:], mu_ps[:], AF.Square,
                                         scale=1.0 / C)
                    var_sb = gp.tile([1, NCH], F32, tag="varrow", bufs=2)
                    nc.vector.scalar_tensor_tensor(
                        var_sb[:], s2_ps[:], 1.0 / C, musq[:],
                        op0=ALU.mult, op1=ALU.subtract)
                    # reciprocal-sigma row (f32r for K=1 matmul use)
                    sqv = gp.tile([1, NCH], F32, tag="sqvrow", bufs=2)
                    nc.scalar.activation(sqv[:], var_sb[:], AF.Sqrt,
                                         bias=eps_col[0:1])
                    nc.vector.reciprocal(sqv[:], sqv[:])
                    rrow = gp.tile([1, NCH], F32R, tag="rrow", bufs=2)
                    rr = rrow[:]
                    nc.scalar.activation(rr, sqv[:], AF.Copy)
                    if r_col is not None:
                        for b in range(NCH // P):
                            blk = (rbase + lo) // P + b
                            rc_ps = pst()
                            nc.tensor.matmul(
                                rc_ps[:, 0:P], rr[:, b * P:(b + 1) * P],
                                ones_row[:], start=True, stop=True)
                            nc.vector.tensor_copy(r_col[:, blk:blk + 1],
                                                  rc_ps[:, 0:1])
                    # broadcast r and mu to all partitions
                    rb_ps = pst()
                    nc.tensor.matmul(rb_ps[:], ones_row[:], rr,
                                     start=True, stop=True)
                    nc.vector.tensor_copy(
                        r_b[:, rbase + lo:rbase + lo + NCH], rb_ps[:])
                    mb_ps = pst()
                    nc.tensor.matmul(mb_ps[:], ones_row[:], mu_sb[:],
                                     start=True, stop=True)
                    nc.vector.tensor_copy(
                        mu_b[:, rbase + lo:rbase + lo + NCH], mb_ps[:])
                    for k in range(CK):
                        nc.vector.tensor_sub(
                            xc[:, k, lo:lo + NCH],
                            src3[:, k, lo:lo + NCH].bitcast(F32),
                            mu_b[:, rbase + lo:rbase + lo + NCH])

            for _rep in range(reps):
                # ===== phase A: LN1 + QKV(self half) + pair exchange + scores =====
                with tc.tile_pool(name="ab", bufs=1) as abp:
                    qT = abp.tile([P, CK, TQ], F32R)
                    sums_ps = [pst(1) for _ in range(TQ // NCH)]
                    with tc.tile_pool(name="pa", bufs=1) as pa:
                        r_b = gp.tile([P, TQ], F32, tag="rb")
                        mu_b = gp.tile([P, TQ], F32, tag="murb")
                        r_col = gp.tile([P, TQ // P], F32, tag="rcol")
                        xt = pa.tile([P, CK, TQ], F32R, tag="xt", bufs=1)
                        for k in range(CK):
                            nc.sync.dma_start(xt[:, k, :], xT3[:, k, :])
                        ln_center(xt, TQ, r_b, mu_b, 0, xt, r_col)
                        xh = xt
                        # k projection (self) -> kv_self[0]
                        for j in range(CK):
                            wblk = pa.tile([P, C], F32R, tag="wqk", bufs=3)
                            nc.sync.dma_start(wblk[:], w1qk[CK + j])
                            for sub in range(TQ // NCH):
                                o_ps = pst()
                                for k in range(CK):
                                    nc.tensor.matmul(
                                        o_ps[:], wblk[:, k * P:(k + 1) * P],
                                        xh[:, k, sub * NCH:(sub + 1) * NCH],
                                        start=(k == 0), stop=(k == CK - 1))
                                kv_sb = pa.tile([P, NCH], F32R, tag="vev",
                                                bufs=4)
                                nc.vector.tensor_mul(
                                    kv_sb[:], o_ps[:],
                                    r_b[:, sub * NCH:(sub + 1) * NCH])
                                nc.vector.tensor_scalar(
                                    kv_sb[:], kv_sb[:].bitcast(F32),
                                    c1k_t[:, j:j + 1], None, op0=ALU.add)
                                nc.sync.dma_start(
                                    kv_self[0, j, :,
                                            sub * NCH:(sub + 1) * NCH],
                                    kv_sb[:])
                        # v projection (self, natural layout) -> kv_self[1]
                        for cc in range(C // NCH):
                            w1vh = pa.tile([P, CK, NCH], F32R, tag="w1vh",
                                           bufs=1)
                            for k in range(CK):
                                nc.sync.dma_start(
                                    w1vh[:, k, :],
                                    w1v[k, :, cc * NCH:(cc + 1) * NCH])
                            for sl in range(TQ // P):
                                v_ps = pst()
                                for k in range(CK):
                                    nc.tensor.matmul(
                                        v_ps[:],
                                        xh[:, k, sl * P:(sl + 1) * P],
                                        w1vh[:, k, :],
                                        start=(k == 0), stop=(k == CK - 1))
                                v_sb = pa.tile([P, NCH], F32R, tag="vev",
                                               bufs=4)
                                nc.vector.scalar_tensor_tensor(
                                    v_sb[:], v_ps[:],
                                    r_col[:, sl:sl + 1],
                                    c1v_t[:, cc * NCH:(cc + 1) * NCH],
                                    op0=ALU.mult, op1=ALU.add)
                                nc.sync.dma_start(
                                    kv_self[1, sl, :,
                                            cc * NCH:(cc + 1) * NCH],
                                    v_sb[:])
                        # pair exchange (overlaps with q projection below)
                        nc.gpsimd.collective_compute(
                            "AllGather", mybir.AluOpType.bypass,
                            replica_groups=[[0, 1], [2, 3], [4, 5], [6, 7]],
                            ins=[kv_self[:]], outs=[kv_all[:]])
                        # q projection (self)
                        for j in range(CK):
                            wblk = pa.tile([P, C], F32R, tag="wqk", bufs=3)
                            nc.sync.dma_start(wblk[:], w1qk[j])
                            for sub in range(TQ // NCH):
                                o_ps = pst()
                                for k in range(CK):
                                    nc.tensor.matmul(
                                        o_ps[:], wblk[:, k * P:(k + 1) * P],
                                        xh[:, k, sub * NCH:(sub + 1) * NCH],
                                        start=(k == 0), stop=(k == CK - 1))
                                dst = qT[:, j, sub * NCH:(sub + 1) * NCH]
                                nc.vector.tensor_mul(
                                    dst, o_ps[:],
                                    r_b[:, sub * NCH:(sub + 1) * NCH])
                                nc.vector.tensor_scalar(
                                    dst, dst.bitcast(F32), c1q_t[:, j:j + 1],
                                    None, op0=ALU.add)
                        # attention scores over both gathered halves
                        for g in range(2):
                            for sl in range(TQ // P):
                                s_idx = g * (TQ // P) + sl
                                kt = pa.tile([P, CK, P], F32R, tag="kt",
                                             bufs=3)
                                nc.sync.dma_start(
                                    kt[:],
                                    kv_all[g, 0, :, :, sl * P:(sl + 1) * P]
                                    .rearrange("k p s -> p k s"))
                                for sub in range(TQ // NCH):
                                    a_ps = pst()
                                    for k in range(CK):
                                        nc.tensor.matmul(
                                            a_ps[:], kt[:, k, :],
                                            qT[:, k,
                                               sub * NCH:(sub + 1) * NCH],
                                            start=(k == 0),
                                            stop=(k == CK - 1))
                                    ae = pa.tile([P, NCH], F32R, tag="vev",
                                                 bufs=4)
                                    nc.scalar.activation(ae[:], a_ps[:],
                                                         AF.Exp,
                                                         scale=ATT_SCALE)
                                    nc.sync.dma_start(
                                        attd[s_idx, :,
                                             sub * NCH:(sub + 1) * NCH],
                                        ae[:])
                                    nc.tensor.matmul(
                                        sums_ps[sub][:], ones_col[:], ae[:],
                                        start=(s_idx == 0),
                                        stop=(s_idx == S - 1))

                # ===== phase C: softmax-normalized y, proj, residual =====
                with tc.tile_pool(name="cd", bufs=1) as cdp:
                    x2T = cdp.tile([P, CK, TQ], F32R)
                    with tc.tile_pool(name="pc", bufs=1) as pc:
                        wp_sb = pc.tile([P, CK, C], F32R, tag="wpb")
                        for sub in range(TQ // NCH):
                            srow = pc.tile([1, NCH], F32, tag="srow",
                                           bufs=2)
                            nc.scalar.activation(srow[:], sums_ps[sub][:],
                                                 AF.Copy)
                            nc.vector.reciprocal(srow[:], srow[:])
                            srow_r = pc.tile([1, NCH], F32R, tag="srowr",
                                             bufs=2)
                            nc.scalar.activation(srow_r[:], srow[:],
                                                 AF.Copy)
                            rb_ps = pst()
                            nc.tensor.matmul(rb_ps[:], ones_row[:],
                                             srow_r[:], start=True,
                                             stop=True)
                            nc.vector.tensor_copy(
                                recip_b[:, sub * NCH:(sub + 1) * NCH],
                                rb_ps[:])
                        yT = pc.tile([P, CK, TQ], F32R, tag="yT")
                        for sub in range(TQ // NCH):
                            y_ps = [pst() for _ in range(CK)]
                            for s in range(S):
                                ar = pc.tile([P, NCH], F32R, tag="ar",
                                             bufs=3)
                                nc.sync.dma_start(
                                    ar[:],
                                    attd[s, :, sub * NCH:(sub + 1) * NCH])
                                va = pc.tile([P, C], F32R, tag="va", bufs=3)
                                nc.sync.dma_start(
                                    va[:], kv_all[s // (TQ // P), 1,
                                                  s % (TQ // P)])
                                if sub == 0 and s < CK:
                                    nc.sync.dma_start(wp_sb[:, s, :], wp[s])
                                for cti in range(CK):
                                    nc.tensor.matmul(
                                        y_ps[cti][:],
                                        va[:, cti * P:(cti + 1) * P],
                                        ar[:], start=(s == 0),
                                        stop=(s == S - 1))
                            for cti in range(CK):
                                nc.vector.tensor_mul(
                                    yT[:, cti, sub * NCH:(sub + 1) * NCH],
                                    y_ps[cti][:],
                                    recip_b[:, sub * NCH:(sub + 1) * NCH])
                        for sub in range(TQ // NCH):
                            for j in range(CK):
                                wpb = wp_sb[:, j]
                                z_ps = pst()
                                for k in range(CK):
                                    nc.tensor.matmul(
                                        z_ps[:], wpb[:, k * P:(k + 1) * P],
                                        yT[:, k, sub * NCH:(sub + 1) * NCH],
                                        start=(k == 0), stop=(k == CK - 1))
                                xq = pc.tile([P, NCH], F32R, tag="xq",
                                             bufs=3)
                                nc.sync.dma_start(
                                    xq[:],
                                    xT3[:, j, sub * NCH:(sub + 1) * NCH])
                                nc.vector.scalar_tensor_tensor(
                                    x2T[:, j, sub * NCH:(sub + 1) * NCH],
                                    z_ps[:], bp_t[:, j:j + 1],
                                    xq[:].bitcast(F32),
                                    op0=ALU.add, op1=ALU.add)
                    if debug_taps:
                        nc.sync.dma_start(yT_d[:], yT[:].bitcast(F32))
                        nc.sync.dma_start(x2_d[:], x2T[:].bitcast(F32))
                    # ===== phase D: LN2 + MLP + final residual =====
                    with tc.tile_pool(name="pd", bufs=1) as pd:
                        r2_b = gp.tile([P, TQ], F32, tag="rb")
                        mu2_b = gp.tile([P, TQ], F32, tag="murb")
                        xh2 = pd.tile([P, CK, TQ], F32R, tag="xh2")
                        ln_center(x2T, TQ, r2_b, mu2_b, 0, xh2)
                        if debug_taps:
                            nc.sync.dma_start(xh2_d[:], xh2[:].bitcast(F32))
                        for hhalf in range(2):
                            gel = pd.tile([P, HK // 2, TQ], F32R,
                                          tag="gel", bufs=1)
                            for jl in range(HK // 2):
                                jh = hhalf * (HK // 2) + jl
                                wb2 = pd.tile([P, C], F32R, tag="wb2",
                                              bufs=2)
                                nc.sync.dma_start(wb2[:], w2[jh])
                                for sub in range(TQ // NCH):
                                    m_ps = pst()
                                    for k in range(CK):
                                        nc.tensor.matmul(
                                            m_ps[:],
                                            wb2[:, k * P:(k + 1) * P],
                                            xh2[:, k,
                                                sub * NCH:(sub + 1) * NCH],
                                            start=(k == 0),
                                            stop=(k == CK - 1))
                                    m1t = pd.tile([P, NCH], F32,
                                                  tag="oev", bufs=3)
                                    nc.vector.tensor_mul(
                                        m1t[:], m_ps[:],
                                        r2_b[:, sub * NCH:(sub + 1) * NCH])
                                    nc.scalar.activation(
                                        gel[:, jl,
                                            sub * NCH:(sub + 1) * NCH],
                                        m1t[:], AF.Gelu_apprx_tanh,
                                        bias=c2_t[:, jh:jh + 1])
                            if debug_taps and hhalf == 0:
                                nc.sync.dma_start(gel_d[:], gel[:].bitcast(F32))
                            for j in range(CK):
                                wmh = pd.tile([P, H // 2], F32R, tag="wmh",
                                              bufs=2)
                                nc.sync.dma_start(
                                    wmh[:],
                                    wm[j, :, hhalf * (H // 2):
                                       (hhalf + 1) * (H // 2)])
                                for sub in range(TQ // NCH):
                                    o_ps = pst()
                                    for kk in range(HK // 2):
                                        nc.tensor.matmul(
                                            o_ps[:],
                                            wmh[:, kk * P:(kk + 1) * P],
                                            gel[:, kk,
                                                sub * NCH:(sub + 1) * NCH],
                                            start=(kk == 0),
                                            stop=(kk == HK // 2 - 1))
                                    o_sb = pd.tile([P, NCH], F32,
                                                   tag="oev", bufs=3)
                                    dst = out_t[j * P:(j + 1) * P,
                                                sub * NCH:(sub + 1) * NCH]
                                    if hhalf == 0:
                                        nc.vector.scalar_tensor_tensor(
                                            o_sb[:], o_ps[:],
                                            bm_t[:, j:j + 1],
                                            x2T[:, j,
                                                sub * NCH:(sub + 1) * NCH]
                                            .bitcast(F32),
                                            op0=ALU.add, op1=ALU.add)
                                        nc.sync.dma_start(dst, o_sb[:])
                                    else:
                                        nc.vector.tensor_copy(o_sb[:],
                                                              o_ps[:])
                                        nc.gpsimd.dma_start(
                                            dst, o_sb[:],
                                            accum_op=ALU.add)
    nc.finalize()
    return nc


_prog = None


def _get_prog():
    global _prog
    if _prog is None:
        _prog = _build()
    return _prog


def _pack_weights(ln1_g, ln1_b, w_attn, b_attn, w_proj, b_proj,
                  ln2_g, ln2_b, w_fc, b_fc, w_mlp_proj, b_mlp_proj):
    f = np.float32
    W1 = (ln1_g[:, None] * w_attn).astype(f)            # [C, 3C]
    c1 = (ln1_b @ w_attn + b_attn).astype(f)            # [3C]
    w1qk = np.ascontiguousarray(
        W1[:, :2 * C].reshape(CK, P, 2 * CK, P).transpose(2, 1, 0, 3)
        .reshape(2 * CK, P, C))
    w1v = np.ascontiguousarray(W1[:, 2 * C:].reshape(CK, P, C))
    wp_t = np.ascontiguousarray(
        w_proj.astype(f).reshape(CK, P, CK, P).transpose(2, 1, 0, 3)
        .reshape(CK, P, C))
    W2 = (ln2_g[:, None] * w_fc).astype(f)              # [C, H]
    c2v = (ln2_b @ w_fc + b_fc).astype(f)               # [H]
    w2_t = np.ascontiguousarray(
        W2.reshape(CK, P, HK, P).transpose(2, 1, 0, 3).reshape(HK, P, C))
    wm_t = np.ascontiguousarray(
        w_mlp_proj.astype(f).reshape(HK, P, CK, P).transpose(2, 1, 0, 3)
        .reshape(CK, P, H))
    return {
        "w1qk": w1qk,
        "w1v": w1v,
        "wp": wp_t,
        "w2": w2_t,
        "wm": wm_t,
        "c1q": np.ascontiguousarray(c1[:C].reshape(CK, P)),
        "c1k": np.ascontiguousarray(c1[C:2 * C].reshape(CK, P)),
        "c1vb": np.ascontiguousarray(
            np.broadcast_to(c1[2 * C:], (P, C)).astype(f)),
        "bp": np.ascontiguousarray(b_proj.astype(f).reshape(CK, P)),
        "c2": np.ascontiguousarray(c2v.reshape(HK, P)),
        "bm": np.ascontiguousarray(b_mlp_proj.astype(f).reshape(CK, P)),
        "onc": np.ones((P, 1), f),
        "onr": np.ones((1, P), f),
    }


def kernel(x, ln1_g, ln1_b, w_attn, b_attn, w_proj, b_proj,
           ln2_g, ln2_b, w_fc, b_fc, w_mlp_proj, b_mlp_proj,
           _trace=False):
    x = np.asarray(x, np.float32)
    shared = _pack_weights(
        np.asarray(ln1_g, np.float32), np.asarray(ln1_b, np.float32),
        np.asarray(w_attn, np.float32), np.asarray(b_attn, np.float32),
        np.asarray(w_proj, np.float32), np.asarray(b_proj, np.float32),
        np.asarray(ln2_g, np.float32), np.asarray(ln2_b, np.float32),
        np.asarray(w_fc, np.float32), np.asarray(b_fc, np.float32),
        np.asarray(w_mlp_proj, np.float32), np.asarray(b_mlp_proj, np.float32))

    in_maps = []
    for core in range(N_CORES):
        b, h = core // 2, core % 2
        xTc = np.ascontiguousarray(x[b, h * TQ:(h + 1) * TQ].T)  # [C, TQ]
        in_maps.append({"xT": xTc, **shared})

    nc = _get_prog()
    res = run_bass_kernel_spmd(nc, in_maps, list(range(N_CORES)),
                               trace=_trace)
    out = np.empty_like(x)
    for core in range(N_CORES):
        b, h = core // 2, core % 2
        out[b, h * TQ:(h + 1) * TQ] = res.results[core]["out_t"].T
    if _trace:
        kernel._last_exec_time_ns = res.exec_time_ns
        kernel._last_profile = res.profile_json
    return out



# revision 11
# speedup vs baseline: 1.2801x; 1.2801x over previous
"""Trainium2 Bass kernel for a dense transformer block (single-head attn + MLP).

v2 design (vs v1 baseline at 651 us):
- No collectives: core c handles batch b=c//2, query-half h=c%2, and
  redundantly computes K,V for ALL 2048 tokens of its batch (the v1 pair
  AllGather measured ~210 us of serial ring time / ~200 us PE idle; the
  duplicate K/V projections cost only ~29 us of PE).
- bf16 matmul operands everywhere (PSUM accumulation stays fp32).  Same PE
  rate as f32r but halves SBUF/DMA, so exp(att) and V stay SBUF-resident
  (no DRAM spill round-trips); K round-trips DRAM (SBUF is tight).
- LayerNorm is never materialized: projections consume RAW x and the
  centering is folded into evictions via the colsum identity
  ((x-mu)*r) @ W = (x@W)*r - (mu*r) * colsum(W).  LN stats matmuls run
  concurrently with projection matmuls, so the PE never waits on stats.
- softmax without max subtraction (|logits| < ~3, verified), denominator
  folded into the y eviction; v-bias folded into the y eviction too.
Host permutes tokens so each core's own query tokens are columns 0..1023
(SPMD uniform program); keys cover all 2048 columns.
"""

import numpy as np
import ml_dtypes
import concourse.bass as bass
import concourse.mybir as mybir
import concourse.tile as tile
from concourse import bacc
from concourse.bass_utils import run_bass_kernel_spmd

F32 = mybir.dt.float32
F32R = mybir.dt.float32r
BF16 = mybir.dt.bfloat16
AF = mybir.ActivationFunctionType
ALU = mybir.AluOpType

P = 128
C = 1024        # n_embd
T = 2048        # key tokens per core (full batch)
TQ = 1024       # query tokens per core
H = 4096        # mlp hidden
CK = C // P     # 8
HK = H // P     # 32
S = T // P      # 16 key tiles
NCH = 512       # matmul moving-dim chunk
EPS = 1e-5
ATT_SCALE = 1.0 / 32.0   # 1/sqrt(C)

N_CORES = 8
BFNP = ml_dtypes.bfloat16


def _build():
    nc = bacc.Bacc()

    xTb = nc.declare_dram_parameter("xTb", [C, T], BF16, isOutput=False)
    xq32 = nc.declare_dram_parameter("xq32", [C, TQ], F32, isOutput=False)
    w1qk = nc.declare_dram_parameter("w1qk", [2 * CK, P, C], BF16,
                                     isOutput=False)
    w1v = nc.declare_dram_parameter("w1v", [CK, P, C], BF16, isOutput=False)
    wp = nc.declare_dram_parameter("wp", [CK, P, C], BF16, isOutput=False)
    w2 = nc.declare_dram_parameter("w2", [HK, P, C], BF16, isOutput=False)
    wm = nc.declare_dram_parameter("wm", [CK, P, H], BF16, isOutput=False)
    # per-output-channel vectors (f32): biases and weight colsums
    c1q = nc.declare_dram_parameter("c1q", [CK, P], F32, isOutput=False)
    c1k = nc.declare_dram_parameter("c1k", [CK, P], F32, isOutput=False)
    cs1q = nc.declare_dram_parameter("cs1q", [CK, P], F32, isOutput=False)
    cs1k = nc.declare_dram_parameter("cs1k", [CK, P], F32, isOutput=False)
    c1v = nc.declare_dram_parameter("c1v", [CK, P], F32, isOutput=False)
    csvb = nc.declare_dram_parameter("csvb", [P, C], F32, isOutput=False)
    bp = nc.declare_dram_parameter("bp", [CK, P], F32, isOutput=False)
    c2 = nc.declare_dram_parameter("c2", [HK, P], F32, isOutput=False)
    cs2 = nc.declare_dram_parameter("cs2", [HK, P], F32, isOutput=False)
    bm = nc.declare_dram_parameter("bm", [CK, P], F32, isOutput=False)
    onc = nc.declare_dram_parameter("onc", [P, 1], BF16, isOutput=False)
    onr = nc.declare_dram_parameter("onr", [1, P], F32R, isOutput=False)
    out_t = nc.declare_dram_parameter("out_t", [C, TQ], F32, isOutput=True)

    ktd = nc.dram_tensor("ktd", [CK, P, T], BF16)

    xT3 = xTb.rearrange("(k p) t -> p k t", p=P)
    xq3 = xq32.rearrange("(k p) t -> p k t", p=P)

    with tile.TileContext(nc) as tc:
        with tc.tile_pool(name="gp", bufs=1) as gp:
            ones_col = gp.tile([P, 1], BF16)
            nc.sync.dma_start(ones_col[:], onc[:])
            ones_row = gp.tile([1, P], F32R)
            nc.sync.dma_start(ones_row[:], onr[:])

            def colvec(name, src, w=CK):
                t = gp.tile([P, w], F32, tag=name)
                nc.sync.dma_start(t[:], src.rearrange("j p -> p j"))
                return t

            c1q_t = colvec("c1q", c1q)
            c1k_t = colvec("c1k", c1k)
            cs1q_t = colvec("cs1q", cs1q)
            cs1k_t = colvec("cs1k", cs1k)
            c1v_t = colvec("c1v", c1v)
            bp_t = colvec("bp", bp)
            bm_t = colvec("bm", bm)
            c2_t = colvec("c2", c2, HK)
            cs2_t = colvec("cs2", cs2, HK)
            csv_t = gp.tile([P, C], F32)
            nc.sync.dma_start(csv_t[:], csvb[:])
            eps_col = gp.tile([P, 1], F32)
            nc.vector.memset(eps_col[:], EPS)

            r_b = gp.tile([P, T], BF16)
            murb = gp.tile([P, T], BF16)      # -(mu*r) broadcast
            r_col = gp.tile([P, S], F32)
            murcol = gp.tile([P, S], F32)     # -(mu*r) per token-slice column
            recip_b = gp.tile([P, TQ], BF16)
            r2_b = gp.tile([P, TQ], BF16)
            murb2 = gp.tile([P, TQ], BF16)
            x2b = gp.tile([P, CK, TQ], BF16)  # attn-sublayer output (bf16)

            def ln_stats(sbp, pp, src3, width, rb, mrb, rcol=None,
                         mrcol=None):
                """Row stats over channels of transposed bf16 activations.
                Fills rb = r (1/sigma) and mrb = -(mu*r) broadcast to all
                partitions; optionally their per-token-slice column forms."""
                for sub in range(width // NCH):
                    lo = sub * NCH
                    mu_ps = pp.tile([1, NCH], F32, tag="ps", bufs=6,
                                    name="mu_ps")
                    s2_ps = pp.tile([1, NCH], F32, tag="ps", bufs=6,
                                    name="s2_ps")
                    for k in range(CK):
                        nc.tensor.matmul(mu_ps[:], ones_col[:],
                                         src3[:, k, lo:lo + NCH],
                                         start=(k == 0), stop=(k == CK - 1))
                    for k in range(CK):
                        sq = sbp.tile([P, NCH], BF16, tag="sq", bufs=2)
                        nc.scalar.activation(sq[:], src3[:, k, lo:lo + NCH],
                                             AF.Square)
                        nc.tensor.matmul(s2_ps[:], ones_col[:], sq[:],
                                         start=(k == 0), stop=(k == CK - 1))
                    mu_row = sbp.tile([1, NCH], F32, tag="murow", bufs=1)
                    nc.scalar.activation(mu_row[:], mu_ps[:], AF.Copy,
                                         scale=1.0 / C)
                    musq = sbp.tile([1, NCH], F32, tag="musq", bufs=1)
                    nc.scalar.activation(musq[:], mu_ps[:], AF.Square,
                                         scale=1.0 / C)
                    sig = sbp.tile([1, NCH], F32, tag="sig", bufs=1)
                    nc.vector.scalar_tensor_tensor(
                        sig[:], s2_ps[:], 1.0 / C, musq[:],
                        op0=ALU.mult, op1=ALU.subtract)
                    nc.scalar.activation(sig[:], sig[:], AF.Sqrt,
                                         bias=eps_col[0:1])
                    nc.vector.reciprocal(sig[:], sig[:])
                    rr = sbp.tile([1, NCH], F32R, tag="rr", bufs=2, name="rr")[:]
                    mnr = sbp.tile([1, NCH], F32R, tag="mnr", bufs=2, name="mnr")[:]
                    nc.scalar.activation(rr, sig[:], AF.Copy)
                    mnrf = sbp.tile([1, NCH], F32, tag="mnrf", bufs=1)
                    nc.vector.scalar_tensor_tensor(
                        mnrf[:], mu_row[:], -1.0, sig[:],
                        op0=ALU.mult, op1=ALU.mult)
                    nc.scalar.activation(mnr, mnrf[:], AF.Copy)
                    for row, dstb in ((rr, rb), (mnr, mrb)):
                        b_ps = pp.tile([P, NCH], F32, tag="ps", bufs=6,
                                       name="b_ps")
                        nc.tensor.matmul(b_ps[:], ones_row[:], row,
                                         start=True, stop=True)
                        nc.vector.tensor_copy(dstb[:, lo:lo + NCH], b_ps[:])
                    if rcol is not None:
                        for blk in range(NCH // P):
                            col = lo // P + blk
                            for row, dstc in ((rr, rcol), (mnr, mrcol)):
                                c_ps = pp.tile([P, NCH], F32, tag="ps",
                                               bufs=6, name="c_ps")
                                nc.tensor.matmul(
                                    c_ps[:, 0:P],
                                    row[:, blk * P:(blk + 1) * P],
                                    ones_row[:], start=True, stop=True)
                                nc.vector.tensor_copy(dstc[:, col:col + 1],
                                                      c_ps[:, 0:1])

            # ===== phase A: stats + QKV + scores (K via DRAM round trip) ===
            with tc.tile_pool(name="h1", bufs=1) as h1:
                va_all = h1.tile([P, S, C], BF16)     # v, token-partitioned
                ar_all = h1.tile([P, S, TQ], BF16)    # exp(att), key-part.
                with (
                    tc.tile_pool(name="px", bufs=1) as px,
                    tc.tile_pool(name="ppa", bufs=1, space="PSUM") as ppa,
                ):
                    qT = px.tile([P, CK, TQ], BF16)
                    xt = px.tile([P, CK, T], BF16)
                    for q4 in range(T // NCH):
                        for k in range(CK):
                            nc.sync.dma_start(
                                xt[:, k, q4 * NCH:(q4 + 1) * NCH],
                                xT3[:, k, q4 * NCH:(q4 + 1) * NCH])
                    ln_stats(px, ppa, xt, T, r_b, murb, r_col, murcol)

                    def proj_evict(dst_bf, o_ps, sub, cs_t, c_t, j, sbp):
                        t1 = sbp.tile([P, NCH], F32, tag="ev1", bufs=3)
                        nc.vector.tensor_mul(
                            t1[:], o_ps[:], r_b[:, sub * NCH:(sub + 1) * NCH])
                        t2 = sbp.tile([P, NCH], F32, tag="ev2", bufs=3)
                        nc.vector.scalar_tensor_tensor(
                            t2[:], murb[:, sub * NCH:(sub + 1) * NCH],
                            cs_t[:, j:j + 1], t1[:],
                            op0=ALU.mult, op1=ALU.add)
                        nc.scalar.activation(dst_bf, t2[:], AF.Identity,
                                             bias=c_t[:, j:j + 1])

                    # k projection (all T tokens) -> ktd (DRAM, bf16)
                    for j in range(CK):
                        wblk = px.tile([P, C], BF16, tag="wqk", bufs=2)
                        nc.sync.dma_start(wblk[:], w1qk[CK + j])
                        for sub in range(T // NCH):
                            o_ps = ppa.tile([P, NCH], F32, tag="ps", bufs=6,
                                            name="o_ps")
                            for k in range(CK):
                                nc.tensor.matmul(
                                    o_ps[:], wblk[:, k * P:(k + 1) * P],
                                    xt[:, k, sub * NCH:(sub + 1) * NCH],
                                    start=(k == 0), stop=(k == CK - 1))
                            kev = px.tile([P, NCH], BF16, tag="kev", bufs=3)
                            proj_evict(kev[:], o_ps, sub, cs1k_t, c1k_t, j,
                                       px)
                            nc.sync.dma_start(
                                ktd[j, :, sub * NCH:(sub + 1) * NCH], kev[:])
                    # q projection (own TQ tokens only) -> qT (SBUF)
                    for j in range(CK):
                        wblk = px.tile([P, C], BF16, tag="wqk", bufs=2)
                        nc.sync.dma_start(wblk[:], w1qk[j])
                        for sub in range(TQ // NCH):
                            o_ps = ppa.tile([P, NCH], F32, tag="ps", bufs=6,
                                            name="o_ps")
                            for k in range(CK):
                                nc.tensor.matmul(
                                    o_ps[:], wblk[:, k * P:(k + 1) * P],
                                    xt[:, k, sub * NCH:(sub + 1) * NCH],
                                    start=(k == 0), stop=(k == CK - 1))
                            proj_evict(qT[:, j, sub * NCH:(sub + 1) * NCH],
                                       o_ps, sub, cs1q_t, c1q_t, j, px)
                    # v projection (all T tokens, natural layout) -> SBUF
                    for cc in range(C // NCH):
                        w1vh = px.tile([P, CK, NCH], BF16, tag="w1vh",
                                       bufs=1)
                        for k in range(CK):
                            nc.sync.dma_start(
                                w1vh[:, k, :],
                                w1v[k, :, cc * NCH:(cc + 1) * NCH])
                        for sl in range(S):
                            v_ps = ppa.tile([P, NCH], F32, tag="ps", bufs=6,
                                            name="v_ps")
                            for k in range(CK):
                                nc.tensor.matmul(
                                    v_ps[:], xt[:, k, sl * P:(sl + 1) * P],
                                    w1vh[:, k, :],
                                    start=(k == 0), stop=(k == CK - 1))
                            vt1 = px.tile([P, NCH], F32, tag="ev1", bufs=3)
                            nc.scalar.activation(vt1[:], v_ps[:], AF.Copy,
                                                 scale=r_col[:, sl:sl + 1])
                            nc.vector.scalar_tensor_tensor(
                                va_all[:, sl, cc * NCH:(cc + 1) * NCH],
                                csv_t[:, cc * NCH:(cc + 1) * NCH],
                                murcol[:, sl:sl + 1], vt1[:],
                                op0=ALU.mult, op1=ALU.add)
                    # attention scores + exp + denominator accumulation
                    sums_ps = [ppa.tile([1, NCH], F32, tag="sums", bufs=2,
                                        name="sums")
                               for _ in range(TQ // NCH)]
                    for sl in range(S):
                        kt = px.tile([P, CK, P], BF16, tag="kt", bufs=3)
                        nc.sync.dma_start(
                            kt[:], ktd[:, :, sl * P:(sl + 1) * P]
                            .rearrange("j p s -> p j s"))
                        for sub in range(TQ // NCH):
                            a_ps = ppa.tile([P, NCH], F32, tag="ps", bufs=6,
                                            name="a_ps")
                            for k in range(CK):
                                nc.tensor.matmul(
                                    a_ps[:], kt[:, k, :],
                                    qT[:, k, sub * NCH:(sub + 1) * NCH],
                                    start=(k == 0), stop=(k == CK - 1))
                            ae = ar_all[:, sl, sub * NCH:(sub + 1) * NCH]
                            nc.scalar.activation(ae, a_ps[:], AF.Exp,
                                                 scale=ATT_SCALE)
                            nc.tensor.matmul(sums_ps[sub][:], ones_col[:],
                                             ae, start=(sl == 0),
                                             stop=(sl == S - 1))
                    # softmax denominators -> reciprocal broadcast
                    for sub in range(TQ // NCH):
                        srow = px.tile([1, NCH], F32, tag="murow", bufs=1,
                                       name="srow")
                        nc.scalar.activation(srow[:], sums_ps[sub][:],
                                             AF.Copy)
                        nc.vector.reciprocal(srow[:], srow[:])
                        srr = px.tile([1, NCH], F32R, tag="rr", bufs=2, name="srr")[:]
                        nc.scalar.activation(srr, srow[:], AF.Copy)
                        rb_ps = ppa.tile([P, NCH], F32, tag="ps", bufs=6,
                                         name="rb_ps")
                        nc.tensor.matmul(rb_ps[:], ones_row[:], srr,
                                         start=True, stop=True)
                        nc.vector.tensor_copy(
                            recip_b[:, sub * NCH:(sub + 1) * NCH], rb_ps[:])

                # ===== phase C: y = softmax @ v, proj, residual =====
                with (
                    tc.tile_pool(name="pc", bufs=1) as pc,
                    tc.tile_pool(name="ppy", bufs=1, space="PSUM") as ppy,
                ):
                    wp_sb = pc.tile([P, CK, C], BF16)
                    for j in range(CK):
                        nc.sync.dma_start(wp_sb[:, j, :], wp[j])
                    yT = pc.tile([P, CK, TQ], BF16)
                    for sub in range(TQ // NCH):
                        y_ps = [ppy.tile([P, NCH], F32, tag="ps", bufs=8,
                                         name="y_ps") for _ in range(CK)]
                        for s in range(S):
                            for cg in range(CK):
                                nc.tensor.matmul(
                                    y_ps[cg][:],
                                    va_all[:, s, cg * P:(cg + 1) * P],
                                    ar_all[:, s, sub * NCH:(sub + 1) * NCH],
                                    start=(s == 0), stop=(s == S - 1))
                        for cg in range(CK):
                            t1 = pc.tile([P, NCH], F32, tag="yev", bufs=3)
                            nc.vector.tensor_mul(
                                t1[:], y_ps[cg][:],
                                recip_b[:, sub * NCH:(sub + 1) * NCH])
                            nc.vector.tensor_scalar(
                                yT[:, cg, sub * NCH:(sub + 1) * NCH], t1[:],
                                c1v_t[:, cg:cg + 1], None, op0=ALU.add)
                        for j in range(CK):
                            z_ps = ppy.tile([P, NCH], F32, tag="ps", bufs=8,
                                            name="z_ps")
                            for k in range(CK):
                                nc.tensor.matmul(
                                    z_ps[:], wp_sb[:, j, k * P:(k + 1) * P],
                                    yT[:, k, sub * NCH:(sub + 1) * NCH],
                                    start=(k == 0), stop=(k == CK - 1))
                            xq_t = pc.tile([P, NCH], F32, tag="xq", bufs=3)
                            nc.sync.dma_start(
                                xq_t[:], xq3[:, j, sub * NCH:(sub + 1) * NCH])
                            t2 = pc.tile([P, NCH], F32, tag="x2f", bufs=3)
                            nc.vector.scalar_tensor_tensor(
                                t2[:], z_ps[:], bp_t[:, j:j + 1], xq_t[:],
                                op0=ALU.add, op1=ALU.add)
                            nc.scalar.activation(
                                x2b[:, j, sub * NCH:(sub + 1) * NCH], t2[:],
                                AF.Copy)

            # ===== phase D: LN2 + MLP + final residual =====
            with (
                tc.tile_pool(name="pd", bufs=1) as pd,
                tc.tile_pool(name="ppd", bufs=1, space="PSUM") as ppd,
            ):
                ln_stats(pd, ppd, x2b, TQ, r2_b, murb2)
                gel = pd.tile([P, HK, TQ], BF16)
                for jh in range(HK):
                    wb2 = pd.tile([P, C], BF16, tag="wb2", bufs=3)
                    nc.sync.dma_start(wb2[:], w2[jh])
                    for sub in range(TQ // NCH):
                        m_ps = ppd.tile([P, NCH], F32, tag="ps", bufs=6,
                                        name="m_ps")
                        for k in range(CK):
                            nc.tensor.matmul(
                                m_ps[:], wb2[:, k * P:(k + 1) * P],
                                x2b[:, k, sub * NCH:(sub + 1) * NCH],
                                start=(k == 0), stop=(k == CK - 1))
                        t1 = pd.tile([P, NCH], F32, tag="ev1", bufs=3)
                        nc.vector.tensor_mul(
                            t1[:], m_ps[:],
                            r2_b[:, sub * NCH:(sub + 1) * NCH])
                        t2 = pd.tile([P, NCH], F32, tag="ev2", bufs=3)
                        nc.vector.scalar_tensor_tensor(
                            t2[:], murb2[:, sub * NCH:(sub + 1) * NCH],
                            cs2_t[:, jh:jh + 1], t1[:],
                            op0=ALU.mult, op1=ALU.add)
                        nc.scalar.activation(
                            gel[:, jh, sub * NCH:(sub + 1) * NCH], t2[:],
                            AF.Gelu_apprx_tanh, bias=c2_t[:, jh:jh + 1])
                for j in range(CK):
                    wmh = pd.tile([P, H], BF16, tag="wmh", bufs=2)
                    nc.sync.dma_start(wmh[:], wm[j])
                    for sub in range(TQ // NCH):
                        o_ps = ppd.tile([P, NCH], F32, tag="ps", bufs=6,
                                        name="o_ps")
                        for kk in range(HK):
                            nc.tensor.matmul(
                                o_ps[:], wmh[:, kk * P:(kk + 1) * P],
                                gel[:, kk, sub * NCH:(sub + 1) * NCH],
                                start=(kk == 0), stop=(kk == HK - 1))
                        o_sb = pd.tile([P, NCH], F32, tag="osb", bufs=3)
                        nc.vector.scalar_tensor_tensor(
                            o_sb[:], o_ps[:], bm_t[:, j:j + 1],
                            x2b[:, j, sub * NCH:(sub + 1) * NCH],
                            op0=ALU.add, op1=ALU.add)
                        nc.sync.dma_start(
                            out_t[j * P:(j + 1) * P,
                                  sub * NCH:(sub + 1) * NCH], o_sb[:])
    nc.finalize()
    return nc


_prog = None


def _get_prog():
    global _prog
    if _prog is None:
        _prog = _build()
    return _prog


def _pack_weights(ln1_g, ln1_b, w_attn, b_attn, w_proj, b_proj,
                  ln2_g, ln2_b, w_fc, b_fc, w_mlp_proj, b_mlp_proj):
    f = np.float32
    W1 = (ln1_g[:, None] * w_attn).astype(BFNP)          # [C, 3C] bf16
    W1f = W1.astype(f)
    c1 = (ln1_b @ W1f + b_attn).astype(f)                # [3C]
    cs1 = W1f.sum(0).astype(f)                           # [3C]
    w1qk = np.ascontiguousarray(
        W1[:, :2 * C].reshape(CK, P, 2 * CK, P).transpose(2, 1, 0, 3)
        .reshape(2 * CK, P, C))
    w1v = np.ascontiguousarray(W1[:, 2 * C:].reshape(CK, P, C))
    wp_t = np.ascontiguousarray(
        w_proj.astype(BFNP).reshape(CK, P, CK, P).transpose(2, 1, 0, 3)
        .reshape(CK, P, C))
    W2 = (ln2_g[:, None] * w_fc).astype(BFNP)            # [C, H] bf16
    W2f = W2.astype(f)
    c2v = (ln2_b @ W2f + b_fc).astype(f)                 # [H]
    cs2v = W2f.sum(0).astype(f)
    w2_t = np.ascontiguousarray(
        W2.reshape(CK, P, HK, P).transpose(2, 1, 0, 3).reshape(HK, P, C))
    wm_t = np.ascontiguousarray(
        w_mlp_proj.astype(BFNP).reshape(HK, P, CK, P).transpose(2, 1, 0, 3)
        .reshape(CK, P, H))
    return {
        "w1qk": w1qk,
        "w1v": w1v,
        "wp": wp_t,
        "w2": w2_t,
        "wm": wm_t,
        "c1q": np.ascontiguousarray(c1[:C].reshape(CK, P)),
        "c1k": np.ascontiguousarray(c1[C:2 * C].reshape(CK, P)),
        "cs1q": np.ascontiguousarray(cs1[:C].reshape(CK, P)),
        "cs1k": np.ascontiguousarray(cs1[C:2 * C].reshape(CK, P)),
        "c1v": np.ascontiguousarray(c1[2 * C:].reshape(CK, P)),
        "csvb": np.ascontiguousarray(
            np.broadcast_to(cs1[2 * C:], (P, C)).astype(f)),
        "bp": np.ascontiguousarray(b_proj.astype(f).reshape(CK, P)),
        "c2": np.ascontiguousarray(c2v.reshape(HK, P)),
        "cs2": np.ascontiguousarray(cs2v.reshape(HK, P)),
        "bm": np.ascontiguousarray(b_mlp_proj.astype(f).reshape(CK, P)),
        "onc": np.ones((P, 1), BFNP),
        "onr": np.ones((1, P), f),
    }


def kernel(x, ln1_g, ln1_b, w_attn, b_attn, w_proj, b_proj,
           ln2_g, ln2_b, w_fc, b_fc, w_mlp_proj, b_mlp_proj,
           _trace=False):
    x = np.asarray(x, np.float32)
    shared = _pack_weights(
        np.asarray(ln1_g, np.float32), np.asarray(ln1_b, np.float32),
        np.asarray(w_attn, np.float32), np.asarray(b_attn, np.float32),
        np.asarray(w_proj, np.float32), np.asarray(b_proj, np.float32),
        np.asarray(ln2_g, np.float32), np.asarray(ln2_b, np.float32),
        np.asarray(w_fc, np.float32), np.asarray(b_fc, np.float32),
        np.asarray(w_mlp_proj, np.float32), np.asarray(b_mlp_proj,
                                                       np.float32))

    in_maps = []
    for core in range(N_CORES):
        b, h = core // 2, core % 2
        xf = x[b].T                                     # [C, T]
        own = xf[:, h * TQ:(h + 1) * TQ]
        other = xf[:, (1 - h) * TQ:(2 - h) * TQ]
        xTb = np.ascontiguousarray(
            np.concatenate([own, other], axis=1)).astype(BFNP)
        xq32 = np.ascontiguousarray(own)
        in_maps.append({"xTb": xTb, "xq32": xq32, **shared})

    nc = _get_prog()
    res = run_bass_kernel_spmd(nc, in_maps, list(range(N_CORES)),
                               trace=_trace)
    out = np.empty_like(x)
    for core in range(N_CORES):
        b, h = core // 2, core % 2
        out[b, h * TQ:(h + 1) * TQ] = res.results[core]["out_t"].T
    if _trace:
        kernel._last_exec_time_ns = res.exec_time_ns
        kernel._last_profile = res.profile_json
    return out


# revision 12
# speedup vs baseline: 1.3548x; 1.0584x over previous
"""Trainium2 Bass kernel for a dense transformer block (single-head attn + MLP).

v3 design (vs v1 baseline at 651 us):
- No collectives: core c handles batch b=c//2, query-half h=c%2, and
  redundantly computes K,V for ALL 2048 tokens of its batch (the v1 pair
  AllGather measured ~210 us of serial ring time / ~200 us PE idle; the
  duplicate K/V projections cost only ~29 us of PE).
- bf16 matmul operands everywhere (PSUM accumulation stays fp32).  Same PE
  rate as f32r but halves SBUF/DMA, so exp(att) and V stay SBUF-resident
  (no DRAM spill round-trips); K round-trips DRAM (SBUF is tight).
- LN stats matmuls (ones-column trick) run on raw x with no PE dependency
  stalls; x is then normalized IN PLACE (two DVE row-broadcast ops per
  chunk) so every projection eviction is a single cheap DVE op and PSUM
  banks release fast.
- softmax without max subtraction (|logits| < ~3, verified), denominator
  and v-bias folded into the y eviction; gelu+bias evicted directly from
  PSUM on the scalar engine; fast Newton reciprocals.
Host permutes tokens so each core's own query tokens are columns 0..1023
(SPMD uniform program); keys cover all 2048 columns.
"""

import numpy as np
import ml_dtypes
import concourse.bass as bass
import concourse.mybir as mybir
import concourse.tile as tile
from concourse import bacc
from concourse.bass_utils import run_bass_kernel_spmd

F32 = mybir.dt.float32
F32R = mybir.dt.float32r
BF16 = mybir.dt.bfloat16
AF = mybir.ActivationFunctionType
ALU = mybir.AluOpType

P = 128
C = 1024        # n_embd
T = 2048        # key tokens per core (full batch)
TQ = 1024       # query tokens per core
H = 4096        # mlp hidden
CK = C // P     # 8
HK = H // P     # 32
S = T // P      # 16 key tiles
NCH = 512       # matmul moving-dim chunk
EPS = 1e-5
ATT_SCALE = 1.0 / 32.0   # 1/sqrt(C)

N_CORES = 8
BFNP = ml_dtypes.bfloat16


def _build():
    nc = bacc.Bacc()

    xTb = nc.declare_dram_parameter("xTb", [C, T], BF16, isOutput=False)
    xq32 = nc.declare_dram_parameter("xq32", [C, TQ], F32, isOutput=False)
    w1qk = nc.declare_dram_parameter("w1qk", [2 * CK, P, C], BF16,
                                     isOutput=False)
    w1v = nc.declare_dram_parameter("w1v", [CK, P, C], BF16, isOutput=False)
    wp = nc.declare_dram_parameter("wp", [CK, P, C], BF16, isOutput=False)
    w2 = nc.declare_dram_parameter("w2", [HK, P, C], BF16, isOutput=False)
    wm = nc.declare_dram_parameter("wm", [CK, P, H], BF16, isOutput=False)
    c1q = nc.declare_dram_parameter("c1q", [CK, P], F32, isOutput=False)
    c1k = nc.declare_dram_parameter("c1k", [CK, P], F32, isOutput=False)
    c1v = nc.declare_dram_parameter("c1v", [CK, P], F32, isOutput=False)
    bp = nc.declare_dram_parameter("bp", [CK, P], F32, isOutput=False)
    c2 = nc.declare_dram_parameter("c2", [HK, P], F32, isOutput=False)
    bm = nc.declare_dram_parameter("bm", [CK, P], F32, isOutput=False)
    onc = nc.declare_dram_parameter("onc", [P, 1], BF16, isOutput=False)
    onr = nc.declare_dram_parameter("onr", [1, P], F32R, isOutput=False)
    out_t = nc.declare_dram_parameter("out_t", [C, TQ], F32, isOutput=True)

    ktd = nc.dram_tensor("ktd", [CK, P, T], BF16)

    xT3 = xTb.rearrange("(k p) t -> p k t", p=P)
    xq3 = xq32.rearrange("(k p) t -> p k t", p=P)

    with tile.TileContext(nc) as tc:
        with tc.tile_pool(name="gp", bufs=1) as gp:
            ones_col = gp.tile([P, 1], BF16)
            nc.sync.dma_start(ones_col[:], onc[:])
            ones_row = gp.tile([1, P], F32R)
            nc.sync.dma_start(ones_row[:], onr[:])

            def colvec(name, src, w=CK):
                t = gp.tile([P, w], F32, tag=name)
                nc.sync.dma_start(t[:], src.rearrange("j p -> p j"))
                return t

            c1q_t = colvec("c1q", c1q)
            c1k_t = colvec("c1k", c1k)
            c1v_t = colvec("c1v", c1v)
            bp_t = colvec("bp", bp)
            bm_t = colvec("bm", bm)
            c2_t = colvec("c2", c2, HK)
            eps_col = gp.tile([P, 1], F32)
            nc.vector.memset(eps_col[:], EPS)

            r_b = gp.tile([P, T], BF16)
            mu_b = gp.tile([P, T], BF16)
            recip_b = gp.tile([P, TQ], BF16)
            r2_b = gp.tile([P, TQ], BF16)
            mu2_b = gp.tile([P, TQ], BF16)
            x2b = gp.tile([P, CK, TQ], BF16)  # attn-sublayer output (bf16)

            def ln_stats(sbp, pp, src3, width, rb, mub):
                """LN row stats over channels of transposed bf16 activations;
                fills rb = 1/sigma and mub = mu broadcast to all partitions."""
                for sub in range(width // NCH):
                    lo = sub * NCH
                    mu_ps = pp.tile([1, NCH], F32, tag="ps", bufs=6,
                                    name="mu_ps")
                    s2_ps = pp.tile([1, NCH], F32, tag="ps", bufs=6,
                                    name="s2_ps")
                    for k in range(CK):
                        nc.tensor.matmul(mu_ps[:], ones_col[:],
                                         src3[:, k, lo:lo + NCH],
                                         start=(k == 0), stop=(k == CK - 1))
                    for k in range(CK):
                        sq = sbp.tile([P, NCH], BF16, tag="sq", bufs=2)
                        nc.scalar.activation(sq[:], src3[:, k, lo:lo + NCH],
                                             AF.Square)
                        nc.tensor.matmul(s2_ps[:], ones_col[:], sq[:],
                                         start=(k == 0), stop=(k == CK - 1))
                    mu_row = sbp.tile([1, NCH], F32, tag="murow", bufs=1)
                    nc.scalar.activation(mu_row[:], mu_ps[:], AF.Copy,
                                         scale=1.0 / C)
                    musq = sbp.tile([1, NCH], F32, tag="musq", bufs=1)
                    nc.scalar.activation(musq[:], mu_ps[:], AF.Square,
                                         scale=1.0 / C)
                    sig = sbp.tile([1, NCH], F32, tag="sig", bufs=1)
                    nc.vector.scalar_tensor_tensor(
                        sig[:], s2_ps[:], 1.0 / C, musq[:],
                        op0=ALU.mult, op1=ALU.subtract)
                    nc.scalar.activation(sig[:], sig[:], AF.Sqrt,
                                         bias=eps_col[0:1])
                    scr = sbp.tile([1, NCH], F32, tag="scr", bufs=1)
                    nc.vector.reciprocal_approx_accurate(sig[:], sig[:],
                                                         scr[:])
                    rr = sbp.tile([1, NCH], F32R, tag="rr", bufs=2,
                                  name="rr")[:]
                    mur = sbp.tile([1, NCH], F32R, tag="mur", bufs=2,
                                   name="mur")[:]
                    nc.scalar.activation(rr, sig[:], AF.Copy)
                    nc.scalar.activation(mur, mu_row[:], AF.Copy)
                    for row, dstb in ((rr, rb), (mur, mub)):
                        b_ps = pp.tile([P, NCH], F32, tag="ps", bufs=6,
                                       name="b_ps")
                        nc.tensor.matmul(b_ps[:], ones_row[:], row,
                                         start=True, stop=True)
                        nc.vector.tensor_copy(dstb[:, lo:lo + NCH], b_ps[:])

            def normalize(dst3, src3, width, rb, mub):
                """dst3 = (src3 - mub) * rb, chunk by chunk (may be
                in-place when dst3 is src3)."""
                for sub in range(width // NCH):
                    lo = sub * NCH
                    for k in range(CK):
                        nc.vector.tensor_sub(dst3[:, k, lo:lo + NCH],
                                             src3[:, k, lo:lo + NCH],
                                             mub[:, lo:lo + NCH])
                        nc.vector.tensor_mul(dst3[:, k, lo:lo + NCH],
                                             dst3[:, k, lo:lo + NCH],
                                             rb[:, lo:lo + NCH])

            # ===== phase A: LN1 + QKV + scores (K via DRAM round trip) =====
            with tc.tile_pool(name="h1", bufs=1) as h1:
                va_all = h1.tile([P, S, C], BF16)     # v, token-partitioned
                ar_all = h1.tile([P, S, TQ], BF16)    # exp(att), key-part.
                with (
                    tc.tile_pool(name="px", bufs=1) as px,
                    tc.tile_pool(name="ppa", bufs=1, space="PSUM") as ppa,
                ):
                    qT = px.tile([P, CK, TQ], BF16)
                    xt = px.tile([P, CK, T], BF16)
                    # prefetch the first k-weight block ahead of x
                    wblk0 = px.tile([P, C], BF16, tag="wqk", bufs=2,
                                    name="wblk0")
                    nc.sync.dma_start(wblk0[:], w1qk[CK])
                    for q4 in range(T // NCH):
                        for k in range(CK):
                            nc.sync.dma_start(
                                xt[:, k, q4 * NCH:(q4 + 1) * NCH],
                                xT3[:, k, q4 * NCH:(q4 + 1) * NCH])
                    ln_stats(px, ppa, xt, T, r_b, mu_b)
                    normalize(xt, xt, T, r_b, mu_b)

                    # k projection (all T tokens) -> ktd (DRAM, bf16)
                    for j in range(CK):
                        if j == 0:
                            wblk = wblk0
                        else:
                            wblk = px.tile([P, C], BF16, tag="wqk", bufs=2,
                                           name="wblk")
                            nc.sync.dma_start(wblk[:], w1qk[CK + j])
                        for sub in range(T // NCH):
                            o_ps = ppa.tile([P, NCH], F32, tag="ps", bufs=6,
                                            name="o_ps")
                            for k in range(CK):
                                nc.tensor.matmul(
                                    o_ps[:], wblk[:, k * P:(k + 1) * P],
                                    xt[:, k, sub * NCH:(sub + 1) * NCH],
                                    start=(k == 0), stop=(k == CK - 1))
                            kev = px.tile([P, NCH], BF16, tag="kev", bufs=3)
                            nc.vector.tensor_scalar(
                                kev[:], o_ps[:], c1k_t[:, j:j + 1], None,
                                op0=ALU.add)
                            nc.sync.dma_start(
                                ktd[j, :, sub * NCH:(sub + 1) * NCH], kev[:])
                    # q projection (own TQ tokens only) -> qT (SBUF)
                    for j in range(CK):
                        wblk = px.tile([P, C], BF16, tag="wqk", bufs=2,
                                       name="wblk")
                        nc.sync.dma_start(wblk[:], w1qk[j])
                        for sub in range(TQ // NCH):
                            o_ps = ppa.tile([P, NCH], F32, tag="ps", bufs=6,
                                            name="o_ps")
                            for k in range(CK):
                                nc.tensor.matmul(
                                    o_ps[:], wblk[:, k * P:(k + 1) * P],
                                    xt[:, k, sub * NCH:(sub + 1) * NCH],
                                    start=(k == 0), stop=(k == CK - 1))
                            nc.vector.tensor_scalar(
                                qT[:, j, sub * NCH:(sub + 1) * NCH],
                                o_ps[:], c1q_t[:, j:j + 1], None,
                                op0=ALU.add)
                    # v projection (all T tokens, natural layout) -> SBUF
                    for cc in range(C // NCH):
                        w1vh = px.tile([P, CK, NCH], BF16, tag="w1vh",
                                       bufs=1)
                        for k in range(CK):
                            nc.sync.dma_start(
                                w1vh[:, k, :],
                                w1v[k, :, cc * NCH:(cc + 1) * NCH])
                        for sl in range(S):
                            v_ps = ppa.tile([P, NCH], F32, tag="ps", bufs=6,
                                            name="v_ps")
                            for k in range(CK):
                                nc.tensor.matmul(
                                    v_ps[:], xt[:, k, sl * P:(sl + 1) * P],
                                    w1vh[:, k, :],
                                    start=(k == 0), stop=(k == CK - 1))
                            nc.vector.tensor_copy(
                                va_all[:, sl, cc * NCH:(cc + 1) * NCH],
                                v_ps[:])
                    # attention scores + exp + denominator accumulation
                    sums_ps = [ppa.tile([1, NCH], F32, tag="sums", bufs=2,
                                        name="sums")
                               for _ in range(TQ // NCH)]
                    for sl in range(S):
                        kt = px.tile([P, CK, P], BF16, tag="kt", bufs=4)
                        nc.sync.dma_start(
                            kt[:], ktd[:, :, sl * P:(sl + 1) * P]
                            .rearrange("j p s -> p j s"))
                        for sub in range(TQ // NCH):
                            a_ps = ppa.tile([P, NCH], F32, tag="ps", bufs=6,
                                            name="a_ps")
                            for k in range(CK):
                                nc.tensor.matmul(
                                    a_ps[:], kt[:, k, :],
                                    qT[:, k, sub * NCH:(sub + 1) * NCH],
                                    start=(k == 0), stop=(k == CK - 1))
                            ae = ar_all[:, sl, sub * NCH:(sub + 1) * NCH]
                            nc.scalar.activation(ae, a_ps[:], AF.Exp,
                                                 scale=ATT_SCALE)
                            nc.tensor.matmul(sums_ps[sub][:], ones_col[:],
                                             ae, start=(sl == 0),
                                             stop=(sl == S - 1))
                    # softmax denominators -> reciprocal broadcast
                    for sub in range(TQ // NCH):
                        srow = px.tile([1, NCH], F32, tag="murow", bufs=1,
                                       name="srow")
                        nc.scalar.activation(srow[:], sums_ps[sub][:],
                                             AF.Copy)
                        scr2 = px.tile([1, NCH], F32, tag="scr", bufs=1,
                                       name="scr2")
                        nc.vector.reciprocal_approx_accurate(srow[:],
                                                             srow[:],
                                                             scr2[:])
                        srr = px.tile([1, NCH], F32R, tag="rr", bufs=2,
                                      name="srr")[:]
                        nc.scalar.activation(srr, srow[:], AF.Copy)
                        rb_ps = ppa.tile([P, NCH], F32, tag="ps", bufs=6,
                                         name="rb_ps")
                        nc.tensor.matmul(rb_ps[:], ones_row[:], srr,
                                         start=True, stop=True)
                        nc.vector.tensor_copy(
                            recip_b[:, sub * NCH:(sub + 1) * NCH], rb_ps[:])

                # ===== phase C: y = softmax @ v, proj, residual =====
                with (
                    tc.tile_pool(name="pc", bufs=1) as pc,
                    tc.tile_pool(name="ppy", bufs=1, space="PSUM") as ppy,
                ):
                    wp_sb = pc.tile([P, CK, C], BF16)
                    for j in range(CK):
                        nc.sync.dma_start(wp_sb[:, j, :], wp[j])
                    yT = pc.tile([P, CK, TQ], BF16)
                    for sub in range(TQ // NCH):
                        y_ps = [ppy.tile([P, NCH], F32, tag="ps", bufs=8,
                                         name="y_ps") for _ in range(CK)]
                        for s in range(S):
                            for cg in range(CK):
                                nc.tensor.matmul(
                                    y_ps[cg][:],
                                    va_all[:, s, cg * P:(cg + 1) * P],
                                    ar_all[:, s, sub * NCH:(sub + 1) * NCH],
                                    start=(s == 0), stop=(s == S - 1))
                        for cg in range(CK):
                            t1 = pc.tile([P, NCH], F32, tag="yev", bufs=3)
                            nc.vector.tensor_mul(
                                t1[:], y_ps[cg][:],
                                recip_b[:, sub * NCH:(sub + 1) * NCH])
                            nc.vector.tensor_scalar(
                                yT[:, cg, sub * NCH:(sub + 1) * NCH], t1[:],
                                c1v_t[:, cg:cg + 1], None, op0=ALU.add)
                        for j in range(CK):
                            z_ps = ppy.tile([P, NCH], F32, tag="ps", bufs=8,
                                            name="z_ps")
                            for k in range(CK):
                                nc.tensor.matmul(
                                    z_ps[:], wp_sb[:, j, k * P:(k + 1) * P],
                                    yT[:, k, sub * NCH:(sub + 1) * NCH],
                                    start=(k == 0), stop=(k == CK - 1))
                            xq_t = pc.tile([P, NCH], F32, tag="xq", bufs=3)
                            nc.sync.dma_start(
                                xq_t[:], xq3[:, j, sub * NCH:(sub + 1) * NCH])
                            nc.vector.scalar_tensor_tensor(
                                x2b[:, j, sub * NCH:(sub + 1) * NCH],
                                z_ps[:], bp_t[:, j:j + 1], xq_t[:],
                                op0=ALU.add, op1=ALU.add)

            # ===== phase D: LN2 + MLP + final residual =====
            with (
                tc.tile_pool(name="pd", bufs=1) as pd,
                tc.tile_pool(name="ppd", bufs=1, space="PSUM") as ppd,
            ):
                # prefetch the first fc weight block
                wb2_0 = pd.tile([P, C], BF16, tag="wb2", bufs=3,
                                name="wb2_0")
                nc.sync.dma_start(wb2_0[:], w2[0])
                # x2n = LN2-normalized copy of x2b (x2b kept for residual)
                x2n = pd.tile([P, CK, TQ], BF16)
                ln_stats(pd, ppd, x2b, TQ, r2_b, mu2_b)
                normalize(x2n, x2b, TQ, r2_b, mu2_b)
                gel = pd.tile([P, HK, TQ], BF16)
                for jh in range(HK):
                    if jh == 0:
                        wb2 = wb2_0
                    else:
                        wb2 = pd.tile([P, C], BF16, tag="wb2", bufs=3,
                                      name="wb2")
                        nc.sync.dma_start(wb2[:], w2[jh])
                    for sub in range(TQ // NCH):
                        m_ps = ppd.tile([P, NCH], F32, tag="ps", bufs=6,
                                        name="m_ps")
                        for k in range(CK):
                            nc.tensor.matmul(
                                m_ps[:], wb2[:, k * P:(k + 1) * P],
                                x2n[:, k, sub * NCH:(sub + 1) * NCH],
                                start=(k == 0), stop=(k == CK - 1))
                        nc.scalar.activation(
                            gel[:, jh, sub * NCH:(sub + 1) * NCH], m_ps[:],
                            AF.Gelu_apprx_tanh, bias=c2_t[:, jh:jh + 1])
                for j in range(CK):
                    wmh = pd.tile([P, H], BF16, tag="wmh", bufs=2)
                    nc.sync.dma_start(wmh[:], wm[j])
                    for sub in range(TQ // NCH):
                        o_ps = ppd.tile([P, NCH], F32, tag="ps", bufs=6,
                                        name="o_ps")
                        for kk in range(HK):
                            nc.tensor.matmul(
                                o_ps[:], wmh[:, kk * P:(kk + 1) * P],
                                gel[:, kk, sub * NCH:(sub + 1) * NCH],
                                start=(kk == 0), stop=(kk == HK - 1))
                        o_sb = pd.tile([P, NCH], F32, tag="osb", bufs=3)
                        nc.vector.scalar_tensor_tensor(
                            o_sb[:], o_ps[:], bm_t[:, j:j + 1],
                            x2b[:, j, sub * NCH:(sub + 1) * NCH],
                            op0=ALU.add, op1=ALU.add)
                        nc.sync.dma_start(
                            out_t[j * P:(j + 1) * P,
                                  sub * NCH:(sub + 1) * NCH], o_sb[:])
    nc.finalize()
    return nc


_prog = None


def _get_prog():
    global _prog
    if _prog is None:
        _prog = _build()
    return _prog


def _pack_weights(ln1_g, ln1_b, w_attn, b_attn, w_proj, b_proj,
                  ln2_g, ln2_b, w_fc, b_fc, w_mlp_proj, b_mlp_proj):
    f = np.float32
    W1 = (ln1_g[:, None] * w_attn).astype(BFNP)          # [C, 3C] bf16
    W1f = W1.astype(f)
    c1 = (ln1_b @ W1f + b_attn).astype(f)                # [3C]
    w1qk = np.ascontiguousarray(
        W1[:, :2 * C].reshape(CK, P, 2 * CK, P).transpose(2, 1, 0, 3)
        .reshape(2 * CK, P, C))
    w1v = np.ascontiguousarray(W1[:, 2 * C:].reshape(CK, P, C))
    wp_t = np.ascontiguousarray(
        w_proj.astype(BFNP).reshape(CK, P, CK, P).transpose(2, 1, 0, 3)
        .reshape(CK, P, C))
    W2 = (ln2_g[:, None] * w_fc).astype(BFNP)            # [C, H] bf16
    W2f = W2.astype(f)
    c2v = (ln2_b @ W2f + b_fc).astype(f)                 # [H]
    w2_t = np.ascontiguousarray(
        W2.reshape(CK, P, HK, P).transpose(2, 1, 0, 3).reshape(HK, P, C))
    wm_t = np.ascontiguousarray(
        w_mlp_proj.astype(BFNP).reshape(HK, P, CK, P).transpose(2, 1, 0, 3)
        .reshape(CK, P, H))
    return {
        "w1qk": w1qk,
        "w1v": w1v,
        "wp": wp_t,
        "w2": w2_t,
        "wm": wm_t,
        "c1q": np.ascontiguousarray(c1[:C].reshape(CK, P)),
        "c1k": np.ascontiguousarray(c1[C:2 * C].reshape(CK, P)),
        "c1v": np.ascontiguousarray(c1[2 * C:].reshape(CK, P)),
        "bp": np.ascontiguousarray(b_proj.astype(f).reshape(CK, P)),
        "c2": np.ascontiguousarray(c2v.reshape(HK, P)),
        "bm": np.ascontiguousarray(b_mlp_proj.astype(f).reshape(CK, P)),
        "onc": np.ones((P, 1), BFNP),
        "onr": np.ones((1, P), f),
    }


def kernel(x, ln1_g, ln1_b, w_attn, b_attn, w_proj, b_proj,
           ln2_g, ln2_b, w_fc, b_fc, w_mlp_proj, b_mlp_proj,
           _trace=False):
    x = np.asarray(x, np.float32)
    shared = _pack_weights(
        np.asarray(ln1_g, np.float32), np.asarray(ln1_b, np.float32),
        np.asarray(w_attn, np.float32), np.asarray(b_attn, np.float32),
        np.asarray(w_proj, np.float32), np.asarray(b_proj, np.float32),
        np.asarray(ln2_g, np.float32), np.asarray(ln2_b, np.float32),
        np.asarray(w_fc, np.float32), np.asarray(b_fc, np.float32),
        np.asarray(w_mlp_proj, np.float32), np.asarray(b_mlp_proj,
                                                       np.float32))

    in_maps = []
    for core in range(N_CORES):
        b, h = core // 2, core % 2
        xf = x[b].T                                     # [C, T]
        own = xf[:, h * TQ:(h + 1) * TQ]
        other = xf[:, (1 - h) * TQ:(2 - h) * TQ]
        xTb = np.ascontiguousarray(
            np.concatenate([own, other], axis=1)).astype(BFNP)
        xq32 = np.ascontiguousarray(own)
        in_maps.append({"xTb": xTb, "xq32": xq32, **shared})

    nc = _get_prog()
    res = run_bass_kernel_spmd(nc, in_maps, list(range(N_CORES)),
                               trace=_trace)
    out = np.empty_like(x)
    for core in range(N_CORES):
        b, h = core // 2, core % 2
        out[b, h * TQ:(h + 1) * TQ] = res.results[core]["out_t"].T
    if _trace:
        kernel._last_exec_time_ns = res.exec_time_ns
        kernel._last_profile = res.profile_json
    return out
